# revision 126
# baseline (speedup 1.0000x reference)
"""Trainium2 Bass kernel for the pre-LN multi-head attention block.

Sharding: 8 cores = 4 batches x 2 query-row halves, collective-free. Each core
computes all 16 heads for its 512 query rows, with full-T k/v for its batch
(k/v compute duplicated across the 2 cores of a batch).

Per-core scheme (C=1024 channels, T=1024 rows, TQ=512 query rows):
  - everything is bf16 into the PE (1 cyc/row + fast weight load); PSUM
    accumulates fp32. Host pre-casts x^T and all weights to bf16.
  - big loads (x chunks, wq slabs, wk 4-slab ring, wv, wp) are pre-triggered
    on the SP hwdge DMA queue as one FIFO so x streams first, then weights.
  - x^T chunks [128, T] are normalized in place (LN stats via bf16
    ones-matmuls on the PE; rstd = exp(-0.5*ln(var+eps)) on ACT; mean/rstd
    rows broadcast across partitions with GpSimd partition_broadcast).
  - q^T / k^T keep channels on partitions; qk-LN gain/bias applied on ACT
    (per-partition scale/bias APs), with stats again from ones-matmuls.
  - v [T, C] is bf16, head-interleaved with a ones column every 65 cols
    (softmax denominator accumulates as row 64 of the av psum).
  - the v matmul loop is interleaved with attention per head-group: v weight
    group g produces heads 4g..4g+3, after which score/exp/av for head pairs
    2g and 2g+1 are emitted. This keeps the ACT queue free of LN-apply work
    by the time the 64 softmax exps (the attention-phase ceiling) start.
  - scores^T per head pair = 2 matmuls (K=64 halves of the chunk); exp on ACT
    (scale=0.125 folded in, no max-subtraction); p stored bf16.
  - attn@v: both heads via 65-col augmented v (bf16), psum pool of 3 banks
    (av double-buffered across head pairs).
  - denominators: psum row 64 -> SBUF (DVE), reciprocal_approx_fast, GpSimd
    partition_broadcast to [64, 2*TQ], then the PSUM drain of av values fuses
    the 1/den scaling (DVE tensor_tensor, banks disjoint from ACT's).
  - proj: y^T = Wp^T out^T + bias from the wp slab prefetched during
    attention; double-buffered psum; host transposes/scatters.
"""

from contextlib import ExitStack

import ml_dtypes
import numpy as np

import concourse.bacc as bacc
import concourse.mybir as mybir
import concourse.tile as tile
from concourse.bass_utils import run_bass_kernel_spmd

F32 = mybir.dt.float32
F32R = mybir.dt.float32r
BF16 = mybir.dt.bfloat16
AF = mybir.ActivationFunctionType
OP = mybir.AluOpType

B, T, C = 4, 1024, 1024
H, D = 16, 64
TQ = 512           # query rows per core
NCH = 8            # 128-row chunks of C (or T)
EPS = 1e-5

_CACHE = {}


def _build():
    nc = bacc.Bacc(None, target_bir_lowering=False, debug=False)

    xT_d = nc.declare_dram_parameter("xT", [C, T], BF16, isOutput=False)
    wq_d = nc.declare_dram_parameter("wq", [C, C], BF16, isOutput=False)
    wk_d = nc.declare_dram_parameter("wk", [C, C], BF16, isOutput=False)
    wv_d = nc.declare_dram_parameter("wv", [C, C], BF16, isOutput=False)
    wp_d = nc.declare_dram_parameter("wp", [C, C], BF16, isOutput=False)
    bq_d = nc.declare_dram_parameter("bq", [C], F32, isOutput=False)
    bk_d = nc.declare_dram_parameter("bk", [C], F32, isOutput=False)
    bv_d = nc.declare_dram_parameter("bv", [C], F32, isOutput=False)
    bp_d = nc.declare_dram_parameter("bp", [C], F32, isOutput=False)
    qg_d = nc.declare_dram_parameter("qg", [C], F32, isOutput=False)
    qb_d = nc.declare_dram_parameter("qb", [C], F32, isOutput=False)
    kg_d = nc.declare_dram_parameter("kg", [C], F32, isOutput=False)
    kb_d = nc.declare_dram_parameter("kb", [C], F32, isOutput=False)
    yT_d = nc.declare_dram_parameter("yT", [C, TQ], F32, isOutput=True)

    with tile.TileContext(nc) as tc, ExitStack() as ctx:
        pool = tc.tile_pool

        const = ctx.enter_context(pool(name="const", bufs=1))
        qsbp = ctx.enter_context(pool(name="qsb", bufs=1))
        ksbp = ctx.enter_context(pool(name="ksb", bufs=1))
        vsbp = ctx.enter_context(pool(name="vsb", bufs=1))
        xzp = ctx.enter_context(pool(name="xz", bufs=1))
        wvp = ctx.enter_context(pool(name="wvpool", bufs=3))

        # temp pools for the LN-stat phases; closed before attention so the
        # attention pools fit in SBUF (pool frees are LIFO: these sit above
        # the whole-kernel pools and below the wk/wq slab pools)
        tmp_ctx = ExitStack()
        bcp = tmp_ctx.enter_context(pool(name="bc", bufs=2))
        rows1 = tmp_ctx.enter_context(pool(name="rows1", bufs=1))
        rows2 = tmp_ctx.enter_context(pool(name="rows2", bufs=2))
        sqp = tmp_ctx.enter_context(pool(name="sq", bufs=2))

        # ---- constants (emitted AFTER the x/weight triggers so the small
        # strided DMAs don't sit ahead of x in the SP hwdge FIFO) ----
        ones_blk = const.tile([128, 128], F32, tag="onesblk")
        nc.vector.memset(ones_blk, 1.0)
        ones1b = const.tile([128, 1], BF16, tag="ones1b")
        nc.vector.tensor_copy(out=ones1b, in_=ones_blk[:, 0:1])
        eps1 = const.tile([1, 1], F32)
        nc.vector.memset(eps1, EPS)

        # persistent activations
        q_sb = qsbp.tile([128, NCH, TQ], BF16)      # q^T, later q-hat
        k_sb = ksbp.tile([128, NCH, T], BF16)       # k^T, later k-hat
        v_sb = vsbp.tile([128, NCH, H * 65], BF16)  # v head-interleaved + ones col

        def ln_rows(pack, srow, n):
            """pack[:, 0:n] = mean, pack[:, n:2n] = rstd from raw [sum|sumsq]
            rows in srow (consumers partition-broadcast from SBUF)."""
            mu = pack[:, 0:n]
            rs = pack[:, n:2 * n]
            ex2 = rows2.tile([1, T], F32, tag="rowtmp")
            nc.vector.tensor_scalar(out=ex2[:, 0:n], in0=srow[:, n:2 * n], scalar1=1.0 / C, scalar2=None, op0=OP.mult)
            mu32 = rows2.tile([1, T], F32, tag="rowmu")
            nc.vector.tensor_scalar(out=mu32[:, 0:n], in0=srow[:, 0:n], scalar1=1.0 / C, scalar2=None, op0=OP.mult)
            nc.vector.tensor_copy(out=mu, in_=mu32[:, 0:n])
            musq = rows2.tile([1, T], F32, tag="rowtmp")
            nc.vector.tensor_tensor(out=musq[:, 0:n], in0=mu32[:, 0:n], in1=mu32[:, 0:n], op=OP.mult)
            nc.vector.tensor_tensor(out=ex2[:, 0:n], in0=ex2[:, 0:n], in1=musq[:, 0:n], op=OP.subtract)
            nc.scalar.activation(out=ex2[:, 0:n], in_=ex2[:, 0:n], func=AF.Ln, bias=eps1, scale=1.0)
            nc.scalar.activation(out=rs, in_=ex2[:, 0:n], func=AF.Exp, scale=-0.5)

        # ============ big-load FIFO: x chunks, then wq, then wk (ACT hwdge
        # queue, pre-triggered so the stream starts at t=0 in this order) ====
        xts = []
        for j in range(NCH):
            t = xzp.tile([128, T], BF16, tag=f"x{j}")
            nc.sync.dma_start(out=t, in_=xT_d[j * 128:(j + 1) * 128, :])
            xts.append(t)

        wk_ctx = ExitStack()
        wkp = wk_ctx.enter_context(pool(name="wkp", bufs=1))
        wq_ctx = ExitStack()
        wqp = wq_ctx.enter_context(pool(name="wqp", bufs=1))

        def wslab(p, wd, m, tag):
            t = p.tile([128, NCH, 128], BF16, tag=tag)
            nc.sync.dma_start(out=t,
                                in_=wd.ap().rearrange("(j p) c -> p j c", p=128)[:, :, m * 128:(m + 1) * 128])
            return t

        wq_tiles = [wslab(wqp, wq_d, m, f"wq{m}") for m in range(NCH)]
        # wk is a 4-slab ring: slabs 4..7 reuse the buffers of 0..3 (triggered
        # inside the k loop once the earlier slab has been consumed)
        wk_tiles = {m: wslab(wkp, wk_d, m, f"wk{m}") for m in range(5)}

        # small consts go behind the big pre-triggered loads in the FIFO
        def vec8(name, d):
            t = const.tile([128, 8], F32, tag=name)
            nc.sync.dma_start(out=t, in_=d.ap().rearrange("(j p) -> p j", p=128))
            return t

        bq8 = vec8("bq8", bq_d)
        bk8 = vec8("bk8", bk_d)
        bp8 = vec8("bp8", bp_d)
        qg8 = vec8("qg8", qg_d)
        qb8 = vec8("qb8", qb_d)
        kg8 = vec8("kg8", kg_d)
        kb8 = vec8("kb8", kb_d)
        bvb = const.tile([128, C], F32)
        nc.sync.dma_start(out=bvb, in_=bv_d.ap().rearrange("c -> () c").to_broadcast([128, C]))

        # ================= phase A: stats, normalize =================
        psA_ctx = ExitStack()
        psA = psA_ctx.enter_context(pool(name="psA", bufs=1, space="PSUM"))
        xstat_ps = psA.tile([1, 2 * T], F32)
        for j in range(NCH):
            sqt = sqp.tile([128, T], BF16, tag="sqb")
            nc.vector.tensor_tensor(out=sqt, in0=xts[j], in1=xts[j], op=OP.mult)
            st, sp = j == 0, j == NCH - 1
            for n in range(2):
                nc.tensor.matmul(xstat_ps[0:1, n * 512:(n + 1) * 512], ones1b,
                                 xts[j][:, n * 512:(n + 1) * 512], start=st, stop=sp)
                nc.tensor.matmul(xstat_ps[0:1, T + n * 512:T + (n + 1) * 512], ones1b,
                                 sqt[:, n * 512:(n + 1) * 512], start=st, stop=sp)
        xpack = rows1.tile([1, 2 * T], BF16, tag="packb")
        ln_rows(xpack, xstat_ps, T)
        psA_ctx.close()
        mub = bcp.tile([128, T], BF16, tag="bcb")
        nc.gpsimd.partition_broadcast(mub, xpack[:, 0:T])
        rsb = bcp.tile([128, T], BF16, tag="bcb")
        nc.gpsimd.partition_broadcast(rsb, xpack[:, T:2 * T])
        for j in range(NCH):
            tz = sqp.tile([128, T], BF16, tag="sqb")
            nc.vector.tensor_tensor(out=tz, in0=xts[j], in1=mub, op=OP.subtract)
            nc.vector.tensor_tensor(out=xts[j], in0=tz, in1=rsb, op=OP.mult)

        # ================= phase B: q / k matmuls + their LNs =================
        qmm_ctx = ExitStack()
        qmmp = qmm_ctx.enter_context(pool(name="qmm", bufs=2, space="PSUM"))

        # --- q ---
        for m in range(NCH):
            wsl = wq_tiles[m]
            q_ps = qmmp.tile([128, T], F32, tag="mm")
            for j in range(NCH):
                nc.tensor.matmul(q_ps[:, 0:TQ], wsl[:, j, :], xts[j][:, 0:TQ],
                                 start=(j == 0), stop=(j == NCH - 1))
            nc.scalar.activation(out=q_sb[:, m, :], in_=q_ps[:, 0:TQ], func=AF.Identity,
                                 bias=bq8[:, m:m + 1], scale=1.0)
        qmm_ctx.close()
        qs_ctx = ExitStack()
        qstatp = qs_ctx.enter_context(pool(name="qstat", bufs=1, space="PSUM"))
        qstat_ps = qstatp.tile([1, 2 * TQ], F32)
        for m in range(NCH):
            sqt = sqp.tile([128, T], BF16, tag="sqb")
            nc.vector.tensor_tensor(out=sqt[:, 0:TQ], in0=q_sb[:, m, :], in1=q_sb[:, m, :], op=OP.mult)
            nc.tensor.matmul(qstat_ps[0:1, 0:TQ], ones1b, q_sb[:, m, :],
                             start=(m == 0), stop=(m == NCH - 1))
            nc.tensor.matmul(qstat_ps[0:1, TQ:2 * TQ], ones1b, sqt[:, 0:TQ],
                             start=(m == 0), stop=(m == NCH - 1))
        qpack = rows1.tile([1, 2 * T], BF16, tag="packb")
        ln_rows(qpack[:, 0:2 * TQ], qstat_ps, TQ)
        qs_ctx.close()
        muqb = bcp.tile([128, T], BF16, tag="bcb")
        nc.gpsimd.partition_broadcast(muqb[:, 0:TQ], qpack[:, 0:TQ])
        rsqb = bcp.tile([128, T], BF16, tag="bcb")
        nc.gpsimd.partition_broadcast(rsqb[:, 0:TQ], qpack[:, TQ:2 * TQ])
        for m in range(NCH):
            t1 = sqp.tile([128, T], BF16, tag="sqb")
            nc.vector.tensor_tensor(out=t1[:, 0:TQ], in0=q_sb[:, m, :], in1=muqb[:, 0:TQ], op=OP.subtract)
            t2 = sqp.tile([128, T], BF16, tag="sqb")
            nc.vector.tensor_tensor(out=t2[:, 0:TQ], in0=t1[:, 0:TQ], in1=rsqb[:, 0:TQ], op=OP.mult)
            nc.scalar.activation(out=q_sb[:, m, :], in_=t2[:, 0:TQ],
                                 func=AF.Identity, bias=qb8[:, m:m + 1],
                                 scale=qg8[:, m:m + 1])
        wq_ctx.close()

        # --- k ---
        kmm_ctx = ExitStack()
        kmmp = kmm_ctx.enter_context(pool(name="kmm", bufs=2, space="PSUM"))
        for m in range(NCH):
            if m + 5 < NCH:
                wk_tiles[m + 5] = wslab(wkp, wk_d, m + 5, f"wk{m}")
            wsl = wk_tiles[m]
            k_ps = kmmp.tile([128, T], F32, tag="mm")
            for n in range(2):
                for j in range(NCH):
                    nc.tensor.matmul(k_ps[:, n * 512:(n + 1) * 512], wsl[:, j, :],
                                     xts[j][:, n * 512:(n + 1) * 512],
                                     start=(j == 0), stop=(j == NCH - 1))
            nc.scalar.activation(out=k_sb[:, m, :], in_=k_ps, func=AF.Identity,
                                 bias=bk8[:, m:m + 1], scale=1.0)

        # v weight slabs queue behind the wk ring in the hwdge FIFO
        wv_tiles = {}

        def trig_wv(g):
            t = wvp.tile([128, NCH, 256], BF16, tag="wv")
            nc.sync.dma_start(
                out=t,
                in_=wv_d.ap().rearrange("(j p) c -> p j c", p=128)[:, :, g * 256:(g + 1) * 256])
            wv_tiles[g] = t

        trig_wv(0)
        trig_wv(1)
        trig_wv(2)

        ks_ctx = ExitStack()
        kstatp = ks_ctx.enter_context(pool(name="kstat", bufs=1, space="PSUM"))
        kstat_ps = kstatp.tile([1, 2 * T], F32)
        for m in range(NCH):
            sqt = sqp.tile([128, T], BF16, tag="sqb")
            nc.vector.tensor_tensor(out=sqt, in0=k_sb[:, m, :], in1=k_sb[:, m, :], op=OP.mult)
            for n in range(2):
                nc.tensor.matmul(kstat_ps[0:1, n * 512:(n + 1) * 512], ones1b,
                                 k_sb[:, m, n * 512:(n + 1) * 512],
                                 start=(m == 0), stop=(m == NCH - 1))
                nc.tensor.matmul(kstat_ps[0:1, T + n * 512:T + (n + 1) * 512], ones1b,
                                 sqt[:, n * 512:(n + 1) * 512],
                                 start=(m == 0), stop=(m == NCH - 1))
        kpack = rows1.tile([1, 2 * T], BF16, tag="packb")
        ln_rows(kpack, kstat_ps, T)
        ks_ctx.close()
        mukb = bcp.tile([128, T], BF16, tag="bcb")
        nc.gpsimd.partition_broadcast(mukb, kpack[:, 0:T])
        rskb = bcp.tile([128, T], BF16, tag="bcb")
        nc.gpsimd.partition_broadcast(rskb, kpack[:, T:2 * T])
        for m in range(NCH):
            t1 = sqp.tile([128, T], BF16, tag="sqb")
            nc.vector.tensor_tensor(out=t1, in0=k_sb[:, m, :], in1=mukb, op=OP.subtract)
            t2 = sqp.tile([128, T], BF16, tag="sqb")
            nc.vector.tensor_tensor(out=t2, in0=t1, in1=rskb, op=OP.mult)
            nc.scalar.activation(out=k_sb[:, m, :], in_=t2,
                                 func=AF.Identity, bias=kb8[:, m:m + 1],
                                 scale=kg8[:, m:m + 1])

        kmm_ctx.close()
        wk_ctx.close()
        tmp_ctx.close()

        # ================= phase C: v interleaved with attention =============
        osbp = ctx.enter_context(pool(name="osb", bufs=1))
        outT_sb = osbp.tile([128, NCH, TQ], BF16)
        wpp = ctx.enter_context(pool(name="wpp", bufs=1))
        youtp = ctx.enter_context(pool(name="yout", bufs=2))
        att_ctx = ExitStack()
        pexpp = att_ctx.enter_context(pool(name="pexp", bufs=8))
        denp = att_ctx.enter_context(pool(name="den", bufs=1))
        rcbp = att_ctx.enter_context(pool(name="rcb", bufs=2))
        vpsp = att_ctx.enter_context(pool(name="vps", bufs=1, space="PSUM"))
        scp = att_ctx.enter_context(pool(name="sc", bufs=2, space="PSUM"))
        avp = att_ctx.enter_context(pool(name="av", bufs=3, space="PSUM"))

        # ones columns of v (emitted here so the slow strided copy stays out
        # of the early DVE queue)
        v_ones_view = v_sb.rearrange("p i (h x) -> p i h x", x=65)[:, :, :, 64:65]
        nc.vector.tensor_copy(out=v_ones_view,
                              in_=ones_blk.rearrange("p (i h x) -> p i h x", i=NCH, h=H))

        wp_sb = wpp.tile([128, NCH, C], BF16)

        p_tiles = {}

        def emit_scores(m):
            p_list = []
            for i in range(NCH):
                sc_ps = scp.tile([128, 1024], F32, tag="sc")
                nc.tensor.matmul(sc_ps[:, 0:512], k_sb[0:64, m, i * 128:(i + 1) * 128],
                                 q_sb[0:64, m, :], start=True, stop=True)
                nc.tensor.matmul(sc_ps[:, 512:1024], k_sb[64:128, m, i * 128:(i + 1) * 128],
                                 q_sb[64:128, m, :], start=True, stop=True)
                p_sb = pexpp.tile([128, 1024], BF16, tag="p")
                nc.scalar.activation(out=p_sb, in_=sc_ps[:, 0:1024], func=AF.Exp, scale=0.125)
                p_list.append(p_sb)
            p_tiles[m] = p_list

        def emit_av(m):
            p_list = p_tiles.pop(m)
            h0, h1 = 2 * m, 2 * m + 1
            av0 = avp.tile([65, TQ], F32, tag="av")
            av1 = avp.tile([65, TQ], F32, tag="av")
            for i in range(NCH):
                st, sp = i == 0, i == NCH - 1
                nc.tensor.matmul(av0, v_sb[:, i, h0 * 65:h0 * 65 + 65],
                                 p_list[i][:, 0:512], start=st, stop=sp)
                nc.tensor.matmul(av1, v_sb[:, i, h1 * 65:h1 * 65 + 65],
                                 p_list[i][:, 512:1024], start=st, stop=sp)
            # denominators: drain PSUM row 64 to SBUF (custom-DVE ops cannot
            # read PSUM), fast approx reciprocal, then one GpSimd
            # partition-broadcast into a base-0 [64, 2*TQ] tile (the ucode
            # ignores a nonzero out partition base)
            dd = denp.tile([1, 2 * TQ], F32, tag="den")
            nc.vector.tensor_copy(out=dd[:, 0:TQ], in_=av0[64:65, :])
            nc.vector.tensor_copy(out=dd[:, TQ:2 * TQ], in_=av1[64:65, :])
            rt = denp.tile([1, 2 * TQ], F32, tag="rect")
            nc.vector.reciprocal_approx_fast(out=rt, in_=dd)
            rbb = rcbp.tile([64, 2 * TQ], F32, tag="rbb")
            nc.gpsimd.partition_broadcast(rbb, rt)
            # drain av values on DVE fused with the 1/den scaling (ACT stays
            # pure-exp during attention; banks are disjoint from score banks)
            nc.vector.tensor_tensor(out=outT_sb[0:64, m, :],
                                    in0=av0[0:64, :], in1=rbb[:, 0:TQ], op=OP.mult)
            nc.vector.tensor_tensor(out=outT_sb[64:128, m, :],
                                    in0=av1[0:64, :], in1=rbb[:, TQ:2 * TQ], op=OP.mult)

        # --- v group g feeds head pairs 2g and 2g+1 ---
        for g in range(4):
            if g == 0:
                trig_wv(3)
            wvsl = wv_tiles[g]
            for i in range(NCH):
                v_ps = vpsp.tile([128, 256], F32, tag="vps")
                for j in range(NCH):
                    nc.tensor.matmul(v_ps, xts[j][:, i * 128:(i + 1) * 128],
                                     wvsl[:, j, :], start=(j == 0), stop=(j == NCH - 1))
                vout = v_sb.rearrange("p i (h x) -> p i h x", x=65)[:, i, g * 4:(g + 1) * 4, 0:64]
                vin = v_ps.rearrange("p (h x) -> p h x", x=64)
                nc.vector.tensor_tensor(
                    out=vout, in0=vin,
                    in1=bvb[:, g * 256:(g + 1) * 256].rearrange("p (h x) -> p h x", x=64),
                    op=OP.add)
            emit_scores(2 * g)
            emit_scores(2 * g + 1)
            emit_av(2 * g)
            emit_av(2 * g + 1)
            if g == 1:
                # proj weight prefetch: queued after the last wv slab so it
                # never delays the attention-critical loads
                for j in range(NCH):
                    nc.sync.dma_start(
                        out=wp_sb[:, j, :],
                        in_=wp_d.ap().rearrange("(j p) c -> p j c", p=128)[:, j, :])

        att_ctx.close()

        # ================= phase D: proj =================
        pjp = ctx.enter_context(pool(name="pj", bufs=2, space="PSUM"))
        for m in range(NCH):
            y_ps = pjp.tile([128, TQ], F32, tag="pj")
            for j in range(NCH):
                nc.tensor.matmul(y_ps, wp_sb[:, j, m * 128:(m + 1) * 128], outT_sb[:, j, :],
                                 start=(j == 0), stop=(j == NCH - 1))
            y_sb = youtp.tile([128, TQ], F32, tag="y")
            nc.scalar.activation(out=y_sb, in_=y_ps, func=AF.Identity,
                                 bias=bp8[:, m:m + 1], scale=1.0)
            nc.sync.dma_start(out=yT_d[m * 128:(m + 1) * 128, :], in_=y_sb)

    nc.finalize()
    return nc


def _get_nc():
    if "nc" not in _CACHE:
        _CACHE["nc"] = _build()
    return _CACHE["nc"]


def _prep_inputs(x, norm_g, norm_b, qkv_w, qkv_b, qln_g, qln_b, kln_g, kln_b, proj_w, proj_b):
    x = np.asarray(x, dtype=np.float32)
    norm_g = np.asarray(norm_g, dtype=np.float32)
    norm_b = np.asarray(norm_b, dtype=np.float32)
    qkv_w = np.asarray(qkv_w, dtype=np.float32)
    qkv_b = np.asarray(qkv_b, dtype=np.float32)

    wfold = norm_g[:, None] * qkv_w                    # [C, 3C]
    bfold = qkv_b + norm_b @ qkv_w                     # [3C]
    wq = np.ascontiguousarray(wfold[:, 0:C])
    wk = np.ascontiguousarray(wfold[:, C:2 * C])
    wv = np.ascontiguousarray(wfold[:, 2 * C:3 * C])
    bq, bk, bv = bfold[0:C].copy(), bfold[C:2 * C].copy(), bfold[2 * C:3 * C].copy()

    bf16 = ml_dtypes.bfloat16
    common = dict(
        wq=wq.astype(bf16), wk=wk.astype(bf16), wv=wv.astype(bf16),
        wp=np.ascontiguousarray(np.asarray(proj_w, dtype=np.float32)).astype(bf16),
        bq=bq, bk=bk, bv=bv,
        bp=np.asarray(proj_b, dtype=np.float32).copy(),
        qg=np.asarray(qln_g, dtype=np.float32).copy(),
        qb=np.asarray(qln_b, dtype=np.float32).copy(),
        kg=np.asarray(kln_g, dtype=np.float32).copy(),
        kb=np.asarray(kln_b, dtype=np.float32).copy(),
    )
    in_maps = []
    for core in range(8):
        b, half = core // 2, core % 2
        xp = np.concatenate([x[b, TQ * half:], x[b, :TQ * half]], axis=0) if half else x[b]
        xT = np.ascontiguousarray(xp.T).astype(bf16)
        in_maps.append(dict(common, xT=xT))
    return in_maps


def kernel(**inputs) -> np.ndarray:
    in_maps = _prep_inputs(**inputs)
    nc = _get_nc()
    res = run_bass_kernel_spmd(nc, in_maps, core_ids=list(range(8)))
    out = np.empty((B, T, C), dtype=np.float32)
    for core in range(8):
        b, half = core // 2, core % 2
        out[b, TQ * half:TQ * half + TQ, :] = res.results[core]["yT"].T
    return out
